# revision 5
# baseline (speedup 1.0000x reference)
"""Elman RNN (B=64, T=512, I=H=512, fp32) on 8 Trainium2 NeuronCores.

Sharding: data-parallel over batch (8 rows/core); weights replicated.
All device-side tensors are kept "transposed" (feature dim on SBUF
partitions, batch on the free dim) so the recurrence needs no on-device
transposes:

  hT_t[m, b] = tanh( UT[m, t, b] + sum_k R[k, m] * hT_{t-1}[k, b] )

with UT = (X @ W + bias)^T precomputed by a batched matmul.

Numerics: PE matmuls run in fp16 (1 cycle/row vs 4 for fp32).  Phase 1
compensates rounding with an x-hi/lo + W-hi/lo split
(U ~ xhi@Whi + xhi@Wlo + xlo@Whi, fp32 PSUM accumulation).  Phase 2 uses
plain fp16 R and fp16 h (validated offline: absmax err ~1.6e-3 vs the
fp32 reference; bf16 would be ~2e-2).
"""

import numpy as np

B, T, I, H = 64, 512, 512, 512
NCORES = 8
BL = B // NCORES  # batch rows per core
P = 128
KT = I // P  # contraction tiles
MT = H // P  # output-feature tiles
CT = 64      # timesteps per chunk (phase-1 granularity)

F16 = np.float16
F32 = np.float32

_CACHE = {}


def _build(t_total):
    import concourse.mybir as mybir
    import concourse.tile as tile
    from concourse import bacc

    dt = mybir.dt
    Tanh = mybir.ActivationFunctionType.Tanh
    n_ch = t_total // CT

    nc = bacc.Bacc("TRN2", target_bir_lowering=False, debug=False,
                   num_devices=NCORES)

    xhi_d = nc.dram_tensor("xhi", [KT, P, t_total * BL], dt.float16,
                           kind="ExternalInput")
    xlo_d = nc.dram_tensor("xlo", [KT, P, t_total * BL], dt.float16,
                           kind="ExternalInput")
    whi_d = nc.dram_tensor("whi", [KT, P, H], dt.float16, kind="ExternalInput")
    wlo_d = nc.dram_tensor("wlo", [KT, P, H], dt.float16, kind="ExternalInput")
    rhi_d = nc.dram_tensor("rhi", [KT, P, H], dt.float16, kind="ExternalInput")
    bias_d = nc.dram_tensor("biasT", [P, MT], dt.float32, kind="ExternalInput")
    h0_d = nc.dram_tensor("h0T", [P, KT * BL], dt.float16,
                          kind="ExternalInput")
    y_d = nc.dram_tensor("yT", [P, t_total, MT, BL], dt.float16,
                         kind="ExternalOutput")

    with tile.TileContext(nc) as tc:
        with (
            tc.tile_pool(name="wp", bufs=1) as wp,
            tc.tile_pool(name="xp", bufs=2) as xp,
            tc.tile_pool(name="up", bufs=3) as up,
            tc.tile_pool(name="yp", bufs=2) as yp,
            tc.tile_pool(name="gp", bufs=2) as gp,
            tc.tile_pool(name="p1", bufs=3, space="PSUM") as p1,
            tc.tile_pool(name="p2", bufs=1, space="PSUM") as p2,
        ):
            # ---- constants ----
            whi_sb = []
            wlo_sb = []
            rhi_sb = []
            for k in range(KT):
                t_ = wp.tile([P, H], dt.float16, tag=f"whi{k}")
                nc.sync.dma_start(t_[:, :], whi_d[k, :, :])
                whi_sb.append(t_)
                t_ = wp.tile([P, H], dt.float16, tag=f"wlo{k}")
                nc.sync.dma_start(t_[:, :], wlo_d[k, :, :])
                wlo_sb.append(t_)
                t_ = wp.tile([P, H], dt.float16, tag=f"rhi{k}")
                nc.sync.dma_start(t_[:, :], rhi_d[k, :, :])
                rhi_sb.append(t_)
            bias_sb = wp.tile([P, MT], dt.float32, tag="bias")
            nc.sync.dma_start(bias_sb[:, :], bias_d[:, :])
            h0_sb = wp.tile([P, KT * BL], dt.float16, tag="h0")
            nc.sync.dma_start(h0_sb[:, :], h0_d[:, :])

            u_tiles = [None] * n_ch

            def phase1_units(c):
                """Generator of small work units computing UT for chunk c."""
                cols = slice(c * CT * BL, (c + 1) * CT * BL)
                xh, xl = [], []
                for k in range(KT):
                    t_ = xp.tile([P, CT * BL], dt.float16, tag=f"xh{k}")
                    nc.sync.dma_start(t_[:, :], xhi_d[k, :, cols])
                    xh.append(t_)
                    yield
                    t_ = xp.tile([P, CT * BL], dt.float16, tag=f"xl{k}")
                    nc.sync.dma_start(t_[:, :], xlo_d[k, :, cols])
                    xl.append(t_)
                    yield
                ut = up.tile([P, MT, CT, BL], dt.float32, tag="ut")
                u_tiles[c] = ut
                for mt in range(MT):
                    msl = slice(mt * P, (mt + 1) * P)
                    ps = p1.tile([P, CT * BL], dt.float32, tag="ps1")
                    for k in range(KT):
                        nc.tensor.matmul(ps[:, :], whi_sb[k][:, msl],
                                         xh[k][:, :], start=(k == 0),
                                         stop=False)
                        yield
                        nc.tensor.matmul(ps[:, :], wlo_sb[k][:, msl],
                                         xh[k][:, :], start=False, stop=False)
                        yield
                        nc.tensor.matmul(ps[:, :], whi_sb[k][:, msl],
                                         xl[k][:, :], start=False,
                                         stop=(k == KT - 1))
                        yield
                    nc.vector.tensor_copy(ut[:, mt, :, :], ps[:, :])
                    yield

            def drain(gen):
                if gen is not None:
                    for _ in gen:
                        pass

            # chunk 0 phase-1 runs as a prologue
            drain(phase1_units(0))

            ych_prev = None
            ych = None
            for c in range(n_ch):
                gen = phase1_units(c + 1) if c + 1 < n_ch else None
                ych_prev = ych
                ych = yp.tile([P, CT, MT, BL], dt.float16, tag="yc")
                for i in range(CT):
                    t = c * CT + i
                    if t == 0:
                        def rhs(k):
                            return h0_sb[:, k * BL:(k + 1) * BL]
                    elif i == 0:
                        def rhs(k, _yp=ych_prev):
                            return _yp[:, CT - 1, k, :]
                    else:
                        def rhs(k, _i=i - 1, _yc=ych):
                            return _yc[:, _i, k, :]
                    for mt in range(MT):
                        msl = slice(mt * P, (mt + 1) * P)
                        ps = p2.tile([P, BL], dt.float32, tag=f"ps2_{mt}")
                        for k in range(KT):
                            nc.tensor.matmul(ps[:, :], rhi_sb[k][:, msl],
                                             rhs(k), start=(k == 0),
                                             stop=(k == KT - 1))
                        g = gp.tile([P, BL], dt.float32, tag=f"g{mt}")
                        nc.vector.tensor_add(g[:, :], ps[:, :],
                                             u_tiles[c][:, mt, i, :])
                        nc.scalar.activation(ych[:, i, mt, :], g[:, :], Tanh,
                                             bias=bias_sb[:, mt:mt + 1])
                    # interleave ~1 unit of next chunk's phase 1 per step
                    if gen is not None:
                        next(gen, None)
                drain(gen)
                nc.sync.dma_start(y_d[:, c * CT:(c + 1) * CT, :, :],
                                  ych[:, :, :, :])
    nc.compile()
    return nc


def _prep_inputs(x, kern, rkern, bias_i, h0, t_total):
    """Host-side sharding + layout/dtype marshalling (no model FLOPs)."""
    def hi_lo(a):
        hi = a.astype(F16)
        lo = (a - hi.astype(F32)).astype(F16)
        return hi, lo

    whi, wlo = hi_lo(kern.reshape(KT, P, H).astype(F32))
    rhi = rkern.reshape(KT, P, H).astype(F16)
    biasT = np.ascontiguousarray(bias_i.reshape(MT, P).T).astype(F32)

    in_maps = []
    for c in range(NCORES):
        xc = x[c * BL:(c + 1) * BL, :t_total, :]          # [BL, T, I]
        xT = np.ascontiguousarray(np.transpose(xc, (2, 1, 0)))  # [I, T, BL]
        xT = xT.reshape(KT, P, t_total * BL)
        xhi, xlo = hi_lo(xT.astype(F32))
        h0c = h0[c * BL:(c + 1) * BL]                     # [BL, H]
        h0T = np.ascontiguousarray(
            h0c.T.reshape(KT, P, BL).transpose(1, 0, 2).reshape(P, KT * BL)
        ).astype(F16)
        in_maps.append({
            "xhi": xhi, "xlo": xlo, "whi": whi, "wlo": wlo,
            "rhi": rhi, "biasT": biasT, "h0T": h0T,
        })
    return in_maps


def _gather(results, t_total):
    outs = np.empty((B, t_total, H), dtype=F32)
    for c in range(NCORES):
        yT = results[c]["yT"].astype(F32)  # [P, T, MT, BL]
        # y[b, t, mt*128+p] = yT[p, t, mt, b]
        outs[c * BL:(c + 1) * BL] = (
            np.transpose(yT, (3, 1, 2, 0)).reshape(BL, t_total, H)
        )
    return outs, np.ascontiguousarray(outs[:, t_total - 1, :])


def kernel(x, kernel, recurrent_kernel, bias_i, h0):
    from concourse.bass_utils import run_bass_kernel_spmd

    x = np.asarray(x, dtype=F32)
    kern = np.asarray(kernel, dtype=F32)
    rkern = np.asarray(recurrent_kernel, dtype=F32)
    bias_i = np.asarray(bias_i, dtype=F32)
    h0 = np.asarray(h0, dtype=F32)

    if "nc" not in _CACHE:
        _CACHE["nc"] = _build(T)
    nc = _CACHE["nc"]
    in_maps = _prep_inputs(x, kern, rkern, bias_i, h0, T)
    res = run_bass_kernel_spmd(nc, in_maps, core_ids=list(range(NCORES)))
    _CACHE["last_result"] = res
    return _gather(res.results, T)


# revision 8
# speedup vs baseline: 4977.5589x; 4977.5589x over previous
"""Elman RNN (B=64, T=512, I=H=512, fp32) on 8 Trainium2 NeuronCores.

Sharding: data-parallel over batch (8 rows/core); weights replicated.
All device-side tensors are kept "transposed" (feature dim on SBUF
partitions, batch on the free dim) so the recurrence needs no on-device
transposes:

  hT_t[m, b] = tanh( UT[m, t, b] + sum_k R[k, m] * hT_{t-1}[k, b] )

with UT = (X @ W + bias)^T precomputed by a batched matmul.

Numerics: PE matmuls run in fp16 (1 cycle/row vs 4 for fp32).  Phase 1
compensates rounding with an x-hi/lo + W-hi/lo split
(U ~ xhi@Whi + xhi@Wlo + xlo@Whi, fp32 PSUM accumulation).  Phase 2 uses
plain fp16 R and fp16 h (validated offline: absmax err ~1.6e-3 vs the
fp32 reference; bf16 would be ~2e-2).
"""

import numpy as np

B, T, I, H = 64, 512, 512, 512
NCORES = 8
BL = B // NCORES  # batch rows per core
P = 128
KT = I // P  # contraction tiles
MT = H // P  # output-feature tiles
CT = 64      # timesteps per chunk (phase-1 granularity)

F16 = np.float16
F32 = np.float32

_CACHE = {}


def _build(t_total, repeats=1):
    import concourse.mybir as mybir
    import concourse.tile as tile
    from concourse import bacc

    dt = mybir.dt
    Tanh = mybir.ActivationFunctionType.Tanh
    n_ch = t_total // CT

    nc = bacc.Bacc("TRN2", target_bir_lowering=False, debug=False,
                   num_devices=NCORES)

    xhi_d = nc.dram_tensor("xhi", [KT, P, t_total * BL], dt.float16,
                           kind="ExternalInput")
    xlo_d = nc.dram_tensor("xlo", [KT, P, t_total * BL], dt.float16,
                           kind="ExternalInput")
    whi_d = nc.dram_tensor("whi", [KT, P, H], dt.float16, kind="ExternalInput")
    wlo_d = nc.dram_tensor("wlo", [KT, P, H], dt.float16, kind="ExternalInput")
    rhi_d = nc.dram_tensor("rhi", [KT, P, H], dt.float16, kind="ExternalInput")
    bias_d = nc.dram_tensor("biasT", [P, MT], dt.float32, kind="ExternalInput")
    h0_d = nc.dram_tensor("h0T", [P, KT * BL], dt.float16,
                          kind="ExternalInput")
    y_d = nc.dram_tensor("yT", [P, t_total, MT, BL], dt.float16,
                         kind="ExternalOutput")

    with tile.TileContext(nc) as tc:
        with (
            tc.tile_pool(name="wp", bufs=1) as wp,
            tc.tile_pool(name="xp", bufs=2) as xp,
            tc.tile_pool(name="up", bufs=3) as up,
            tc.tile_pool(name="yp", bufs=2) as yp,
            tc.tile_pool(name="gp", bufs=2) as gp,
            tc.tile_pool(name="p1", bufs=3, space="PSUM") as p1,
            tc.tile_pool(name="p2", bufs=1, space="PSUM") as p2,
        ):
            # ---- constants ----
            whi_sb = []
            wlo_sb = []
            rhi_sb = []
            for k in range(KT):
                t_ = wp.tile([P, H], dt.float16, tag=f"whi{k}")
                nc.sync.dma_start(t_[:, :], whi_d[k, :, :])
                whi_sb.append(t_)
                t_ = wp.tile([P, H], dt.float16, tag=f"wlo{k}")
                nc.sync.dma_start(t_[:, :], wlo_d[k, :, :])
                wlo_sb.append(t_)
                t_ = wp.tile([P, H], dt.float16, tag=f"rhi{k}")
                nc.sync.dma_start(t_[:, :], rhi_d[k, :, :])
                rhi_sb.append(t_)
            bias_sb = wp.tile([P, MT], dt.float32, tag="bias")
            nc.sync.dma_start(bias_sb[:, :], bias_d[:, :])
            h0_sb = wp.tile([P, KT * BL], dt.float16, tag="h0")
            nc.sync.dma_start(h0_sb[:, :], h0_d[:, :])

            for _rep in range(repeats):
                _emit_body(nc, tc, dt, Tanh, t_total, n_ch, xhi_d, xlo_d, y_d,
                           whi_sb, wlo_sb, rhi_sb, bias_sb, h0_sb,
                           xp, up, yp, gp, p1, p2)
    nc.compile()
    return nc


def _emit_body(nc, tc, dt, Tanh, t_total, n_ch, xhi_d, xlo_d, y_d,
               whi_sb, wlo_sb, rhi_sb, bias_sb, h0_sb,
               xp, up, yp, gp, p1, p2):
    if True:
        if True:
            u_tiles = [None] * n_ch

            def phase1_units(c):
                """Generator of small work units computing UT for chunk c."""
                cols = slice(c * CT * BL, (c + 1) * CT * BL)
                xh, xl = [], []
                for k in range(KT):
                    t_ = xp.tile([P, CT * BL], dt.float16, tag=f"xh{k}")
                    nc.sync.dma_start(t_[:, :], xhi_d[k, :, cols])
                    xh.append(t_)
                    yield
                    t_ = xp.tile([P, CT * BL], dt.float16, tag=f"xl{k}")
                    nc.sync.dma_start(t_[:, :], xlo_d[k, :, cols])
                    xl.append(t_)
                    yield
                ut = up.tile([P, MT, CT, BL], dt.float32, tag="ut")
                u_tiles[c] = ut
                for mt in range(MT):
                    msl = slice(mt * P, (mt + 1) * P)
                    ps = p1.tile([P, CT * BL], dt.float32, tag="ps1")
                    for k in range(KT):
                        nc.tensor.matmul(ps[:, :], whi_sb[k][:, msl],
                                         xh[k][:, :], start=(k == 0),
                                         stop=False)
                        yield
                        nc.tensor.matmul(ps[:, :], wlo_sb[k][:, msl],
                                         xh[k][:, :], start=False, stop=False)
                        yield
                        nc.tensor.matmul(ps[:, :], whi_sb[k][:, msl],
                                         xl[k][:, :], start=False,
                                         stop=(k == KT - 1))
                        yield
                    nc.vector.tensor_copy(ut[:, mt, :, :], ps[:, :])
                    yield

            def drain(gen):
                if gen is not None:
                    for _ in gen:
                        pass

            # chunk 0 phase-1 runs as a prologue
            drain(phase1_units(0))

            ych_prev = None
            ych = None
            for c in range(n_ch):
                gen = phase1_units(c + 1) if c + 1 < n_ch else None
                ych_prev = ych
                ych = yp.tile([P, CT, MT, BL], dt.float16, tag="yc")
                for i in range(CT):
                    t = c * CT + i
                    if t == 0:
                        def rhs(k):
                            return h0_sb[:, k * BL:(k + 1) * BL]
                    elif i == 0:
                        def rhs(k, _yp=ych_prev):
                            return _yp[:, CT - 1, k, :]
                    else:
                        def rhs(k, _i=i - 1, _yc=ych):
                            return _yc[:, _i, k, :]
                    for mt in range(MT):
                        msl = slice(mt * P, (mt + 1) * P)
                        ps = p2.tile([P, BL], dt.float32, tag=f"ps2_{mt}")
                        for k in range(KT):
                            nc.tensor.matmul(ps[:, :], rhi_sb[k][:, msl],
                                             rhs(k), start=(k == 0),
                                             stop=(k == KT - 1))
                        g = gp.tile([P, BL], dt.float32, tag=f"g{mt}")
                        nc.vector.tensor_add(g[:, :], ps[:, :],
                                             u_tiles[c][:, mt, i, :])
                        nc.scalar.activation(ych[:, i, mt, :], g[:, :], Tanh,
                                             bias=bias_sb[:, mt:mt + 1])
                    # interleave ~1 unit of next chunk's phase 1 per step
                    if gen is not None:
                        next(gen, None)
                drain(gen)
                nc.sync.dma_start(y_d[:, c * CT:(c + 1) * CT, :, :],
                                  ych[:, :, :, :])


def _prep_inputs(x, kern, rkern, bias_i, h0, t_total):
    """Host-side sharding + layout/dtype marshalling (no model FLOPs)."""
    def hi_lo(a):
        hi = a.astype(F16)
        lo = (a - hi.astype(F32)).astype(F16)
        return hi, lo

    whi, wlo = hi_lo(kern.reshape(KT, P, H).astype(F32))
    rhi = rkern.reshape(KT, P, H).astype(F16)
    biasT = np.ascontiguousarray(bias_i.reshape(MT, P).T).astype(F32)

    in_maps = []
    for c in range(NCORES):
        xc = x[c * BL:(c + 1) * BL, :t_total, :]          # [BL, T, I]
        xT = np.ascontiguousarray(np.transpose(xc, (2, 1, 0)))  # [I, T, BL]
        xT = xT.reshape(KT, P, t_total * BL)
        xhi, xlo = hi_lo(xT.astype(F32))
        h0c = h0[c * BL:(c + 1) * BL]                     # [BL, H]
        h0T = np.ascontiguousarray(
            h0c.T.reshape(KT, P, BL).transpose(1, 0, 2).reshape(P, KT * BL)
        ).astype(F16)
        in_maps.append({
            "xhi": xhi, "xlo": xlo, "whi": whi, "wlo": wlo,
            "rhi": rhi, "biasT": biasT, "h0T": h0T,
        })
    return in_maps


def _gather(results, t_total):
    outs = np.empty((B, t_total, H), dtype=F32)
    for c in range(NCORES):
        yT = results[c]["yT"].astype(F32)  # [P, T, MT, BL]
        # y[b, t, mt*128+p] = yT[p, t, mt, b]
        outs[c * BL:(c + 1) * BL] = (
            np.transpose(yT, (3, 1, 2, 0)).reshape(BL, t_total, H)
        )
    return outs, np.ascontiguousarray(outs[:, t_total - 1, :])


def kernel(x, kernel, recurrent_kernel, bias_i, h0):
    from concourse.bass_utils import run_bass_kernel_spmd

    x = np.asarray(x, dtype=F32)
    kern = np.asarray(kernel, dtype=F32)
    rkern = np.asarray(recurrent_kernel, dtype=F32)
    bias_i = np.asarray(bias_i, dtype=F32)
    h0 = np.asarray(h0, dtype=F32)

    if "nc" not in _CACHE:
        _CACHE["nc"] = _build(T)
    nc = _CACHE["nc"]
    in_maps = _prep_inputs(x, kern, rkern, bias_i, h0, T)
    res = run_bass_kernel_spmd(nc, in_maps, core_ids=list(range(NCORES)))
    _CACHE["last_result"] = res
    return _gather(res.results, T)
